# revision 8
# baseline (speedup 1.0000x reference)
"""Trainium2 Bass kernel for nn_Model_39676907882504.

Math: qk = (q @ k^T)/8 has shape [1,2048,1,1]; after the transposes it is
[2048,1,1,1], and softmax over the trailing size-1 axis is exactly 1.0
regardless of qk.  The final matmul with attn_weight == 1 reduces to
broadcasting `value` across a new leading dim:

    output[i, j, 0, :] = value[0, j, 0, :]   for all i in [0, 2048)

i.e. a 512KB -> 1GiB broadcast copy.  Pure memory-regime kernel.
Sharding: 256 output rows per core x 8 cores; value replicated.

HW model (trace analysis + engine-assignment probes):
  - HWDGE assigns descriptor position i of EVERY instruction to SDMA
    engine 64+(i%16), restarting at 64 each instruction.
  - SBUF AXI port p serves partitions ≡ p (mod 16).  An instruction whose
    descriptors walk CONTIGUOUS partitions is a port<->engine bijection
    (shifted by start%16): no two engines share a port.
  - Engines 64-78 sustain ~26.9 GB/s each; engine 79 only ~21.4 GB/s
    (known silicon quirk).  Instruction length L gives engine e
    ceil((L-e)/16) descriptors, so L ≡ 11..13 (mod 16) under-weights the
    high engines (incl. 79) and the busiest engines are 0..k.
  - Instructions with only 1 descriptor per engine serialize at ~5.4us
    per descriptor (completion latency un-overlapped); >=4-8 descs per
    engine stream at port rate.  So: few BIG instructions.

Kernel: value is cut into 15 overlapping windows per output row
(window p = vflat[8738p : 8738p+8740]; adjacent descriptors overlap by
2 floats - same data, harmless).  SBUF holds the window set replicated
8x across 120 partitions (partition q = window q%15), so window w of row
r sits at partition (15r+w) mod 120 and any whole-row span is a
CONTIGUOUS partition run.  Stores go in whole-row pieces of 5 rows (75
descs) + 3 rows (45 descs) per 8-row cycle: engine loads per cycle are
{e0-10: 8, e11-12: 7, e13-15: 6} - slow engine 79 gets 6/8 of uniform,
ports stay aligned, packets stay big.  256 rows split over both HWDGE
queues (sync + scalar).
"""

import sys

for _p in ("/opt/trn_rl_repo",):
    if _p not in sys.path:
        sys.path.insert(0, _p)

import numpy as np

import bass_rust
import concourse.bass as bass
import concourse.mybir as mybir
from concourse.bass_utils import run_bass_kernel_spmd

S = 2048
D = 64
N_CORES = 8
ROWS_PER_CORE = S // N_CORES          # 256
ROW_FL = S * D                        # 131072 floats per output row
NW = 15                               # windows per row
C = 8740                              # floats per window (34960 B)
STRIDE = 8738                         # in-row stride between windows
assert (NW - 1) * STRIDE + C == ROW_FL
NPART = NW * 8                        # 120 partitions of window replicas
ROW_B = ROW_FL * 4
# whole-row store pieces per 8-row cycle (in rows)
PIECES = (5, 3)
assert sum(PIECES) * NW == NPART

TRACE = False          # test.py flips this to profile
TRACE_KWARGS = {}
LAST_RESULT = None     # BassKernelResults of the last run (for test.py)


def build_program():
    nc = bass.Bass()
    # partitions 0-119: window q%15 of the row; partition 120: 2-float tail
    # vflat[131070:131072] (so the 16-desc closer writes real data).
    val = nc.declare_dram_parameter("value_w", [NPART + 1, C],
                                    mybir.dt.float32, isOutput=False)
    out = nc.declare_dram_parameter("out", [ROWS_PER_CORE, ROW_FL],
                                    mybir.dt.float32, isOutput=True)
    wtile = nc.alloc_sbuf_tensor("wtile", [NPART + 1, C], mybir.dt.float32)

    def store_piece(eng, row0, g):
        """one instruction: rows row0..row0+g-1 (g*15 descs, contiguous
        partitions starting at (15*row0)%120)"""
        p0 = (NW * row0) % NPART
        o = out[row0:row0 + g, 0:ROW_FL]
        o.ap = bass_rust.VecI64Pair([[ROW_FL, g], [STRIDE, NW], [1, C]])
        i = wtile[p0:p0 + g * NW, 0:C]   # one contiguous partition run
        return eng.dma_start(out=o, in_=i)

    def closer(eng, r):
        # 16 x 1-float rewrite of row r positions {8738p}: descriptor j
        # reads partition 105+j (windows 0-14 then the tail partition),
        # all 16 engines covered, FIFO-behind every earlier descriptor.
        o = out[r:r + 1, 0:ROW_FL]
        o.ap = bass_rust.VecI64Pair([[STRIDE, 16], [1, 1]])
        with nc.allow_non_contiguous_dma(reason="16 x 4B queue-drain marker"):
            return eng.dma_start(out=o, in_=wtile[NPART - NW:NPART + 1, 0:1])

    half = ROWS_PER_CORE // 2

    with nc.Block() as block, nc.semaphore("dma_sem") as dma_sem, \
            nc.semaphore("scr_sem") as scr_sem:

        @block.sync
        def _(sync):
            sync.dma_start(out=wtile[:, :], in_=val[:, :]).then_inc(dma_sem, 16)
            sync.wait_ge(dma_sem, 16)
            r = 0
            while r < half:
                for g in PIECES:
                    store_piece(sync, r, g).then_inc(scr_sem, 16)
                    r += g
            closer(sync, 0).then_inc(dma_sem, 16)
            sync.wait_ge(dma_sem, 48)

        @block.scalar
        def _(scalar):
            scalar.wait_ge(dma_sem, 16)
            r = half
            while r < ROWS_PER_CORE:
                for g in PIECES:
                    store_piece(scalar, r, g).then_inc(scr_sem, 16)
                    r += g
            closer(scalar, half).then_inc(dma_sem, 16)
            scalar.wait_ge(dma_sem, 48)

    return nc


def _pack_value(value):
    vflat = np.ascontiguousarray(np.asarray(value, np.float32)).reshape(ROW_FL)
    vw = np.zeros((NPART + 1, C), np.float32)
    for p in range(NW):
        vw[p] = vflat[p * STRIDE: p * STRIDE + C]
    for k in range(1, 8):
        vw[k * NW:(k + 1) * NW] = vw[0:NW]
    tail = vflat[NW * STRIDE:]
    vw[NPART, :tail.size] = tail
    return vw


def kernel(query=None, key=None, value=None, attn_mask=None, **_ignored):
    global LAST_RESULT
    vw = _pack_value(value)

    nc = build_program()
    core_ids = list(range(N_CORES))
    in_maps = [{"value_w": vw} for _ in core_ids]
    res = run_bass_kernel_spmd(nc, in_maps, core_ids, trace=TRACE,
                               **TRACE_KWARGS)
    LAST_RESULT = res

    shards = [res.results[i]["out"].reshape(ROWS_PER_CORE, S, 1, D)
              for i in range(N_CORES)]
    return np.concatenate(shards, axis=0)


# revision 9
# speedup vs baseline: 2.6888x; 2.6888x over previous
"""Trainium2 Bass kernel for nn_Model_39676907882504.

Math: qk = (q @ k^T)/8 has shape [1,2048,1,1]; after the transposes it is
[2048,1,1,1], and softmax over the trailing size-1 axis is exactly 1.0
regardless of qk.  The final matmul with attn_weight == 1 reduces to
broadcasting `value` across a new leading dim:

    output[i, j, 0, :] = value[0, j, 0, :]   for all i in [0, 2048)

i.e. a 512KB -> 1GiB broadcast copy.  Pure memory-regime kernel.
Sharding: 256 output rows per core x 8 cores; value replicated in SBUF.

HW model (established by trace analysis + probe kernels this session):
  - A dynamic DMA instruction is split into PACKETS, one per index of the
    DRAM-side AP's outermost dim; packet i goes to SDMA engine 64+(i%16),
    restarting at 64 for every instruction.
  - Packets of 1 descriptor serialize at ~5.4us each (completion latency
    un-overlapped); packets with >=7-15 descriptors stream at the SBUF
    port rate (~26.9 GB/s per engine, 32B x 850MHz = 27.2 peak).
  - SBUF AXI port p serves partitions ≡ p (mod 16).  A packet covers a
    contiguous run of G partitions; engine j walks partitions Gj..Gj+G-1,
    so G must be coprime to 16 or engines j, j+16/gcd collide on ports.
  - Engine 79 (index 15) only sustains ~21.4 GB/s (known silicon quirk);
    every other engine does ~26.9.
  - An instruction's SBUF AP partition dim is dim 0 and is limited to
    <=128 partitions, so packets x descs <= 128.

Kernel: each output row (131072 floats) is written by ONE instruction in
one of two shapes, both with 7-descriptor packets (7 coprime 16: no port
aliasing; big enough to stream):

  alpha: 16 packets x 7 descs = 112 overlapping windows of 1202 floats,
         stride 1170 (111*1170+1202 = 131072).  All 16 engines, uniform.
  gamma: 15 packets x 7 descs = 105 windows of 1280 floats, stride 1248
         (104*1248+1280 = 131072).  Engines 64-78 only; engine 79 idle.

A ~4:1 alpha:gamma mix loads engine 79 at ~80% of uniform, matching its
~21.4/26.9 speed ratio, so all 16 engines finish together (~324us of
stores vs 434us baseline).  Window overlaps rewrite identical bytes -
harmless.  256 rows split across both HWDGE queues (sync + scalar).
"""

import sys

for _p in ("/opt/trn_rl_repo",):
    if _p not in sys.path:
        sys.path.insert(0, _p)

import numpy as np

import bass_rust
import concourse.bass as bass
import concourse.mybir as mybir
from concourse.bass_utils import run_bass_kernel_spmd

S = 2048
D = 64
N_CORES = 8
ROWS_PER_CORE = S // N_CORES          # 256
ROW_FL = S * D                        # 131072 floats per output row

# alpha shape: 16 packets x 7 descs
NA, SA, CA = 112, 1170, 1202
assert (NA - 1) * SA + CA == ROW_FL
# gamma shape: 15 packets x 7 descs (engine 79 idle)
NG, SG, CG = 105, 1248, 1280
assert (NG - 1) * SG + CG == ROW_FL
# gamma every GAMMA_EVERY-th row (per queue) ~= 1/5 of rows
GAMMA_EVERY = 5

TRACE = False          # test.py flips this to profile
TRACE_KWARGS = {}
LAST_RESULT = None     # BassKernelResults of the last run (for test.py)


def build_program():
    nc = bass.Bass()
    va = nc.declare_dram_parameter("value_a", [NA, CA], mybir.dt.float32,
                                   isOutput=False)
    vg = nc.declare_dram_parameter("value_g", [NG, CG], mybir.dt.float32,
                                   isOutput=False)
    out = nc.declare_dram_parameter("out", [ROWS_PER_CORE, ROW_FL],
                                    mybir.dt.float32, isOutput=True)
    wta = nc.alloc_sbuf_tensor("wta", [NA, CA], mybir.dt.float32)
    wtg = nc.alloc_sbuf_tensor("wtg", [NG, CG], mybir.dt.float32)

    def store_row(eng, r):
        o = out[r:r + 1, 0:ROW_FL]
        if r % GAMMA_EVERY == GAMMA_EVERY - 1:
            o.ap = bass_rust.VecI64Pair([[7 * SG, 15], [SG, 7], [1, CG]])
            return eng.dma_start(out=o, in_=wtg[0:NG, 0:CG])
        o.ap = bass_rust.VecI64Pair([[7 * SA, 16], [SA, 7], [1, CA]])
        return eng.dma_start(out=o, in_=wta[0:NA, 0:CA])

    def closer(eng, r):
        # full alpha re-store of an already-written row: touches all 16
        # engines, FIFO-behind every earlier packet on this queue.
        o = out[r:r + 1, 0:ROW_FL]
        o.ap = bass_rust.VecI64Pair([[7 * SA, 16], [SA, 7], [1, CA]])
        return eng.dma_start(out=o, in_=wta[0:NA, 0:CA])

    half = ROWS_PER_CORE // 2

    with nc.Block() as block, nc.semaphore("dma_sem") as dma_sem, \
            nc.semaphore("scr_sem") as scr_sem:

        @block.sync
        def _(sync):
            sync.dma_start(out=wta[:, :], in_=va[:, :]).then_inc(dma_sem, 16)
            sync.dma_start(out=wtg[:, :], in_=vg[:, :]).then_inc(dma_sem, 16)
            sync.wait_ge(dma_sem, 32)
            for r in range(0, half):
                store_row(sync, r).then_inc(scr_sem, 16)
            closer(sync, 0).then_inc(dma_sem, 16)
            sync.wait_ge(dma_sem, 64)

        @block.scalar
        def _(scalar):
            scalar.wait_ge(dma_sem, 32)
            for r in range(half, ROWS_PER_CORE):
                store_row(scalar, r).then_inc(scr_sem, 16)
            closer(scalar, half).then_inc(dma_sem, 16)
            scalar.wait_ge(dma_sem, 64)

    return nc


def _windows(vflat, n, stride, c):
    w = np.zeros((n, c), np.float32)
    for p in range(n):
        w[p] = vflat[p * stride: p * stride + c]
    return w


def kernel(query=None, key=None, value=None, attn_mask=None, **_ignored):
    global LAST_RESULT
    vflat = np.ascontiguousarray(np.asarray(value, np.float32)).reshape(ROW_FL)
    va = _windows(vflat, NA, SA, CA)
    vg = _windows(vflat, NG, SG, CG)

    nc = build_program()
    core_ids = list(range(N_CORES))
    in_maps = [{"value_a": va, "value_g": vg} for _ in core_ids]
    res = run_bass_kernel_spmd(nc, in_maps, core_ids, trace=TRACE,
                               **TRACE_KWARGS)
    LAST_RESULT = res

    shards = [res.results[i]["out"].reshape(ROWS_PER_CORE, S, 1, D)
              for i in range(N_CORES)]
    return np.concatenate(shards, axis=0)
